# revision 65
# baseline (speedup 1.0000x reference)
"""BarrierNet forward pass on 8 Trainium2 NeuronCores (pure data parallel).

Network (per sample, batch 8192 sharded 1024/core):
  x[5] -> 1024 -> 1024 -> {512, 512} -> {512, 512} -> two 2-wide heads
  followed by a closed-form single-constraint QP projection (dCBF barrier).

v2: fp8(e4m3) DoubleRow matmuls for L2..L4 + heads (~105us -> ~65us vs
the f32r v1). End-to-end rel err ~1.9e-3 (gate 2e-2). Key mechanics:
  - Weights quantized per-tensor with power-of-2 scales chosen so each
    layer's PSUM comes out already in the next layer's storage scale:
    the PSUM->SBUF step is a single add-bias/relu/cast-fp8 instruction
    (Vector tensor_scalar or Scalar activation), no rescale pass.
  - That forces near-unity weight scales; fp8 subnormal storage of the
    uniform-init weights costs ~3.1% rms vs 2.7% at full range (the PE
    widens fp8 to e6m3 internally, honoring subnormals — HW-verified).
  - L1 stays f32r (K=5; bf16/fp8-DR are SLOWER there) with alpha1 folded
    into W1 host-side; activation scales calibrated from a host fp32
    forward of the actual batch (margin 192/240 — never clips to Inf).
  - Layer-major schedule: both 512-sample batch tiles run per weight
    chunk so each stationary is loaded once; the tile-1 LDWEIGHTS that
    the legalizer emits anyway is deleted post-schedule (the PE weight
    array persists across matmuls).
  - The sigmoid branch (L32/L42/head) runs before the identity branch so
    its ACT-table swap (sin and sigmoid share no table) and head output
    processing hide under ~11us of remaining matmuls.
  - Heads run DoubleRow (weights padded to 32 cols); the un-scaling
    1/(beta5*alpha4) enters via ACT scale APs, so the compiled program
    has no data-dependent immediates.
Layout per core: feature-major [feat, batch] tiles, BT=512 batch tiles,
DVE 32x32 stream-transpose for the heads, QP/barrier epilogue on
[32, 32] strided views (batched across both tiles) on Vector/Scalar.
"""

import numpy as np

import concourse.bass as bass
import concourse.tile as tile
from concourse import bacc, mybir
from concourse.bass_utils import run_bass_kernel_spmd

N_CORES = 8
B_FULL = 8192
BC = B_FULL // N_CORES      # batch per core
BT = 512                    # batch tile (matmul moving free dim)
NBT = BC // BT              # batch tiles per core
GPB = BT // 32              # 32-sample groups per batch tile (16)

D1, D2, D3, D4 = 1024, 1024, 512, 512
KP2, KP3, KP4, KP5 = D1 // 256, D2 // 256, D3 // 256, D4 // 256  # k-pair counts
N1, N2, N3, N4 = D1 // 128, D2 // 128, D3 // 128, D4 // 128      # out chunks
L1C, L2C, OBS_X, OBS_Y, RADIUS = 3.0, 3.0, 0.0, 7.0, 4.0

MARGIN = 192.0              # fp8 activation headroom (max normal 240)

F32 = mybir.dt.float32
F32R = mybir.dt.float32r
FP8 = mybir.dt.float8e4
AF = mybir.ActivationFunctionType
AL = mybir.AluOpType
DR = mybir.MatmulPerfMode.DoubleRow

# bias_pack column offsets per layer
BOF = {"l1": 0, "l2": 8, "l31": 16, "l32": 20, "l41": 24, "l42": 28}


def build_program(consts):
    """Build the SPMD Bass program.
    consts = (mean[5], std[5], ml[2], sl[2])."""
    mean, std, ml, sl = consts

    nc = bacc.Bacc("TRN2", target_bir_lowering=False, debug=False,
                   num_devices=N_CORES)

    def din(name, shape, dt):
        return nc.dram_tensor(name, shape, dt, kind="ExternalInput").ap()

    xw_d = din("xw", [5, BC + D1], F32R)  # xT and alpha1*W1 in one DMA
    W2_d = din("W2p", [128, KP2 * 2 * D2], FP8)
    W31_d = din("W31p", [128, KP3 * 2 * D3], FP8)
    W32_d = din("W32p", [128, KP3 * 2 * D3], FP8)
    W41_d = din("W41p", [128, KP4 * 2 * D4], FP8)
    W42_d = din("W42p", [128, KP4 * 2 * D4], FP8)
    W5_d = din("W5p", [128, 2 * KP5 * 2 * 32], FP8)
    Xep_d = din("Xep", [32, NBT * GPB * 5], F32)
    bias_d = din("biasp", [128, 32], F32)
    hb_d = din("hbp", [2, 4], F32)   # cols: b51, b52, 1/(b51sc), 1/(b52sc)
    out_d = nc.dram_tensor("out", [32, NBT * GPB * 2], F32,
                           kind="ExternalOutput").ap()

    with tile.TileContext(nc) as tc:
        with (
            tc.tile_pool(name="wpool", bufs=1) as wp,
            tc.tile_pool(name="acts", bufs=28) as ap_,
            tc.tile_pool(name="misc", bufs=1) as mp,
            tc.tile_pool(name="ep", bufs=1) as ep,
            tc.tile_pool(name="pmm", bufs=8, space="PSUM") as pmm,
        ):
            # ---- input/weight loads -------------------------------------
            # small L1 operands + biases first on the sync ring so the first
            # matmul isn't queued behind the 1MB W2 transfer
            def sync_load(dram, shape, tg, dt=FP8, pool=None):
                t = (pool or wp).tile(shape, dt, tag=tg, name=f"{tg}_t")
                nc.sync.dma_start(out=t, in_=dram)
                return t

            xw = sync_load(xw_d, [5, BC + D1], "xw", F32R, pool=mp)
            xT, w1 = xw[:, 0:BC], xw[:, BC:BC + D1]
            biasp = sync_load(bias_d, [128, 32], "biasp", F32, pool=mp)
            w2 = sync_load(W2_d, [128, KP2 * 2 * D2], "w2")
            w31 = sync_load(W31_d, [128, KP3 * 2 * D3], "w31")
            w32 = sync_load(W32_d, [128, KP3 * 2 * D3], "w32")
            w41 = sync_load(W41_d, [128, KP4 * 2 * D4], "w41")
            w42 = sync_load(W42_d, [128, KP4 * 2 * D4], "w42")

            def gp_load(dram, shape, tg, dt=F32):
                t = mp.tile(shape, dt, tag=tg, name=f"{tg}_t")
                nc.gpsimd.dma_start(out=t, in_=dram)
                return t

            Xep = gp_load(Xep_d, [32, NBT * GPB * 5], "Xep")
            w5 = gp_load(W5_d, [128, 2 * KP5 * 2 * 32], "w5", FP8)
            hb = gp_load(hb_d, [2, 4], "hb")
            phd = pmm  # heads share the 8-bank PSUM pool

            OUT = mp.tile([32, NBT * GPB * 2], F32, tag="OUT", name="OUT_t")

            # weight views: [128, pairs, plane, N]
            w2v = w2.rearrange("p (t i n) -> p t i n", t=KP2, i=2)
            w31v = w31.rearrange("p (t i n) -> p t i n", t=KP3, i=2)
            w32v = w32.rearrange("p (t i n) -> p t i n", t=KP3, i=2)
            w41v = w41.rearrange("p (t i n) -> p t i n", t=KP4, i=2)
            w42v = w42.rearrange("p (t i n) -> p t i n", t=KP4, i=2)
            w5v = w5.rearrange("p (h t i c) -> p h t i c", h=2, t=KP5, i=2)

            _cbias_cache = {}

            def cbias(val, parts):
                val = float(val)
                if val not in _cbias_cache:
                    t = ep.tile([128, 1], F32, tag=f"cb{len(_cbias_cache)}",
                                name=f"cb{len(_cbias_cache)}")
                    nc.vector.memset(t, val)
                    _cbias_cache[val] = t
                return _cbias_cache[val][0:parts, :]

            def eact(out, in_, func, bias=0.0, scale=1.0):
                if isinstance(bias, float) and func not in (AF.Copy,):
                    bias = cbias(bias, in_.shape[0])
                nc.scalar.activation(out, in_, func, bias=bias, scale=scale)

            def store_act(dst, ps, bcol, n):
                """dst(fp8) = relu(psum + bias): single instruction. The two
                batch tiles of a chunk go to different engines so the psum
                pair drains in parallel."""
                if n % 2 == 0:
                    nc.vector.tensor_scalar(dst, ps, bcol, 0.0, AL.add, AL.max)
                else:
                    nc.scalar.activation(dst, ps, AF.Relu, bias=bcol)

            HPI = float(np.pi / 2)
            PI = float(np.pi)

            def epilogue_pre():
                """x-only QP/barrier quantities for ALL batch tiles at once
                ([32, NBT*GPB] ops); runs on Vector/Scalar while the PE is
                in the dense layers."""
                NF = NBT * GPB
                Xv = Xep.rearrange("p (f j) -> p f j", j=5)

                def T(nm):
                    return ep.tile([32, NF], F32, tag=nm, bufs=1,
                                   name=f"{nm}_pre")

                def emul(o, a, b):
                    nc.vector.tensor_mul(o, a, b)

                def eadd(o, a, b):
                    nc.vector.tensor_add(o, a, b)

                def stt(o, a, s, op0, b, op1):
                    nc.vector.scalar_tensor_tensor(o, a, float(s), b, op0, op1)

                t1r, w1r = Xv[:, :, 0], Xv[:, :, 1]
                t2r, w2r = Xv[:, :, 2], Xv[:, :, 3]

                if float(std[0]) == 1.0 and float(mean[0]) == 0.0:
                    t1m = t1r
                else:
                    t1m = T("t1m"); eact(t1m, t1r, AF.Copy, bias=float(mean[0]), scale=float(std[0]))
                if float(std[2]) == 1.0 and float(mean[2]) == 0.0:
                    t2m = t2r
                else:
                    t2m = T("t2m"); eact(t2m, t2r, AF.Copy, bias=float(mean[2]), scale=float(std[2]))

                def sincos(theta, nm):
                    ws = T(nm + "_ws"); nc.vector.add_range_wrap(ws, theta, 0.0, PI, 2 * PI)
                    s = T(nm + "_s"); eact(s, ws, AF.Sin)
                    wc = T(nm + "_wc"); nc.vector.add_range_wrap(wc, theta, HPI, PI, 2 * PI)
                    c = T(nm + "_c"); eact(c, wc, AF.Sin)
                    return s, c

                s1, c1 = sincos(t1m, "t1")
                s2, c2 = sincos(t2m, "t2")

                if float(std[1]) == 1.0 and float(mean[1]) == 0.0:
                    w1v_ = w1r
                else:
                    w1v_ = T("w1v"); eact(w1v_, w1r, AF.Copy, bias=float(mean[1]), scale=float(std[1]))
                if float(std[3]) == 1.0 and float(mean[3]) == 0.0:
                    w2v_ = w2r
                else:
                    w2v_ = T("w2v"); eact(w2v_, w2r, AF.Copy, bias=float(mean[3]), scale=float(std[3]))

                pxu = T("pxu"); eadd(pxu, c1, c2)
                px = T("px"); eact(px, pxu, AF.Copy, bias=-OBS_X, scale=L1C)
                pyu = T("pyu"); eadd(pyu, s1, s2)
                py = T("py"); eact(py, pyu, AF.Copy, bias=-OBS_Y, scale=L1C)

                a1 = T("a1"); emul(a1, s1, w1v_)
                a2 = T("a2"); emul(a2, s2, w2v_)
                vxn = T("vxn"); eadd(vxn, a1, a2)          # = -vx/3
                bb1 = T("bb1"); emul(bb1, c1, w1v_)
                bb2 = T("bb2"); emul(bb2, c2, w2v_)
                vyu = T("vyu"); eadd(vyu, bb1, bb2)
                vy = T("vy"); eact(vy, vyu, AF.Copy, scale=3.0)

                q1 = T("q1"); emul(q1, px, vxn)
                q2 = T("q2"); emul(q2, py, vy)
                bdot2 = T("bdot2"); stt(bdot2, q1, -3.0, AL.mult, q2, AL.add)

                w1sq = T("w1sq"); emul(w1sq, w1v_, w1v_)
                w2sq = T("w2sq"); emul(w2sq, w2v_, w2v_)
                cw1 = T("cw1"); emul(cw1, c1, w1sq)
                cw2 = T("cw2"); emul(cw2, c2, w2sq)
                cw = T("cw"); eadd(cw, cw1, cw2)
                sw1 = T("sw1"); emul(sw1, s1, w1sq)
                sw2 = T("sw2"); emul(sw2, s2, w2sq)
                sw = T("sw"); eadd(sw, sw1, sw2)
                t1x = T("t1x"); emul(t1x, px, cw)
                t2y = T("t2y"); emul(t2y, py, sw)
                txy = T("txy"); eadd(txy, t1x, t2y)
                vv1 = T("vv1"); emul(vv1, vxn, vxn)
                vv2 = T("vv2"); emul(vv2, vy, vy)
                vv = T("vv"); stt(vv, vv1, 9.0, AL.mult, vv2, AL.add)
                Lhalf = T("Lhalf"); stt(Lhalf, txy, -3.0, AL.mult, vv, AL.add)

                g1a = T("g1a"); emul(g1a, px, s1)
                g1b = T("g1b"); emul(g1b, py, c1)
                g2a = T("g2a"); emul(g2a, px, s2)
                g2b = T("g2b"); emul(g2b, py, c2)
                G12 = ep.tile([32, NF * 2], F32, tag="G12", bufs=1,
                              name="G12_pre")
                G12v = G12.rearrange("p (f q) -> p f q", q=2)
                G1h, G2h = G12v[:, :, 0], G12v[:, :, 1]
                stt(G1h, g1b, -1.0, AL.mult, g1a, AL.add)  # G1/6
                stt(G2h, g2b, -1.0, AL.mult, g2a, AL.add)  # G2/6

                pxsq = T("pxsq"); emul(pxsq, px, px)
                pysq = T("pysq"); emul(pysq, py, py)
                bar = T("bar"); stt(bar, pxsq, -RADIUS * RADIUS, AL.add, pysq, AL.add)

                d1 = T("d1"); emul(d1, G1h, G1h)
                d2 = T("d2"); emul(d2, G2h, G2h)
                den36 = T("den36"); stt(den36, d1, 1e-12 / 36.0, AL.add, d2, AL.add)
                nrec = T("nrec"); nc.vector.reciprocal(nrec, den36)

                return dict(bdot2=bdot2, bar=bar, Lhalf=Lhalf,
                            G1h=G1h, G2h=G2h, G12=G12, nrec=nrec)

            def post_early(vtb, pre):
                """Sigmoid-dependent half of the QP tail, both tiles at once
                ([32, 32] ops): runs under the identity branch's matmuls."""
                NF = NBT * GPB
                Yvb = vtb.rearrange("p (f q) -> p f q", q=32)
                sg1, sg2 = Yvb[:, :, 0], Yvb[:, :, 1]

                def T(nm):
                    return ep.tile([32, NF], F32, tag=nm, bufs=1,
                                   name=f"{nm}_pearly")

                ssum = T("ssum"); nc.vector.tensor_add(ssum, sg1, sg2)
                sprod = T("sprod"); nc.vector.tensor_mul(sprod, sg1, sg2)
                hb_ = T("hb_"); nc.vector.tensor_mul(hb_, ssum, pre["bdot2"])
                hc = T("hc"); nc.vector.tensor_mul(hc, sprod, pre["bar"])
                va2 = T("va2"); nc.vector.scalar_tensor_tensor(
                    va2, hc, 8.0, pre["Lhalf"], AL.mult, AL.add)
                va = T("va"); nc.vector.scalar_tensor_tensor(
                    va, hb_, 4.0, va2, AL.mult, AL.add)  # h/2
                return va

            def epilogue_post(bt, vta, va_all, pre):
                """Identity-head-dependent tail of the QP for batch tile bt."""
                fsl = slice(bt * GPB, (bt + 1) * GPB)
                Yva = vta.rearrange("p (f q) -> p f q", q=32)[:, fsl, :]
                OUTv = OUT.rearrange("p (f i) -> p f i", i=2)[:, fsl, :]

                def T(nm):
                    return ep.tile([32, GPB], F32, tag=nm, bufs=NBT,
                                   name=f"{nm}_post{bt}")

                def emul(o, a, b):
                    nc.vector.tensor_mul(o, a, b)

                def eadd(o, a, b):
                    nc.vector.tensor_add(o, a, b)

                def stt(o, a, s, op0, b, op1):
                    nc.vector.scalar_tensor_tensor(o, a, float(s), b, op0, op1)

                nrec = pre["nrec"][:, fsl]
                va = va_all[:, fsl]
                G12s = pre["G12"].rearrange("p (f q) -> p f q", q=2)[:, fsl, :]
                P12 = Yva[:, :, 0:2]  # [32, GPB, 2]

                r12 = ep.tile([32, GPB * 2], F32, tag="r12", bufs=NBT,
                              name=f"r12_post{bt}")
                r12v = r12.rearrange("p (f q) -> p f q", q=2)
                nc.vector.tensor_mul(r12v, G12s, P12)
                rs = T("rs"); eadd(rs, r12v[:, :, 0], r12v[:, :, 1])
                vb = T("vb"); stt(vb, rs, 3.0, AL.mult, va, AL.add)    # viol=-2vb

                vr = T("vr")
                nc.vector.tensor_scalar(vr, vb, -1.0, 0.0, AL.mult, AL.max)
                lam18 = T("lam18"); emul(lam18, vr, nrec)

                lam18b = bass.AP(tensor=lam18.tensor, offset=lam18.offset,
                                 ap=list(lam18.ap) + [[0, 2]])
                lg12 = ep.tile([32, GPB * 2], F32, tag="lg12", bufs=NBT,
                               name=f"lg12_post{bt}")
                lg12v = lg12.rearrange("p (f q) -> p f q", q=2)
                nc.vector.tensor_mul(lg12v, lam18b, G12s)
                if (float(sl[0]) == 1.0 and float(sl[1]) == 1.0
                        and float(ml[0]) == 0.0 and float(ml[1]) == 0.0):
                    # out = -(lg12/3 + P12): one DVE op straight into OUT
                    stt(OUTv[:, :, 0:2], lg12v, -1.0 / 3.0, AL.mult, P12,
                        AL.subtract)
                else:
                    u12n = ep.tile([32, GPB * 2], F32, tag="u12n", bufs=NBT,
                                   name=f"u12n_post{bt}")
                    u12v = u12n.rearrange("p (f q) -> p f q", q=2)
                    stt(u12v, lg12v, 1.0 / 3.0, AL.mult, P12, AL.add)
                    eact(OUTv[:, :, 0], u12v[:, :, 0], AF.Copy,
                         bias=-float(ml[0]) / float(sl[0]),
                         scale=-1.0 / float(sl[0]))
                    eact(OUTv[:, :, 1], u12v[:, :, 1], AF.Copy,
                         bias=-float(ml[1]) / float(sl[1]),
                         scale=-1.0 / float(sl[1]))

            def pair_tiles(nm, n_pairs, bt):
                return [ap_.tile([128, 2 * BT], FP8, tag="act",
                                 name=f"{nm}_p{t}b{bt}")
                        for t in range(n_pairs)]

            def layer1():
                """L1 (f32r, K=5, alpha1 pre-folded into W1) -> fp8 pairs.
                Tile-outer so tile 0's stores drain while tile 1's matmuls
                run — L2 (which needs ALL of a tile's x1) starts sooner."""
                x1p = [pair_tiles("x1", N1 // 2, bt) for bt in range(NBT)]
                for bt in range(NBT):
                    for n in range(N1):
                        ps = pmm.tile([128, BT], F32, tag="pm",
                                      name=f"ps1_{n}b{bt}")
                        nc.tensor.matmul(
                            ps, w1[:, n * 128:(n + 1) * 128],
                            xT[:, bt * BT:(bt + 1) * BT], start=True,
                            stop=True)
                        store_act(
                            x1p[bt][n // 2][:, (n % 2) * BT:(n % 2 + 1) * BT],
                            ps, biasp[:, BOF["l1"] + n:BOF["l1"] + n + 1], n)
                return x1p

            def dense_dr(nm, inp, wv, n_pairs_k, n_out, bof):
                """fp8 DoubleRow dense layer, both batch tiles per stationary
                (tile 1 reuses the loaded weights: ldweights=False)."""
                outp = [pair_tiles(nm, n_out // 2, bt) for bt in range(NBT)]
                for n in range(n_out):
                    ps = [pmm.tile([128, BT], F32, tag="pm",
                                   name=f"ps{nm}_{n}b{bt}")
                          for bt in range(NBT)]
                    for t in range(n_pairs_k):
                        for bt in range(NBT):
                            rhs = inp[bt][t].rearrange("p (i b) -> p i b", i=2)
                            r = nc.tensor.matmul(
                                ps[bt], wv[:, t, :, n * 128:(n + 1) * 128],
                                rhs, start=(t == 0),
                                stop=(t == n_pairs_k - 1), perf_mode=DR)
                            if bt > 0:
                                r.ins.ldweights = False
                    for bt in range(NBT):
                        store_act(
                            outp[bt][n // 2][:, (n % 2) * BT:(n % 2 + 1) * BT],
                            ps[bt], biasp[:, bof + n:bof + n + 1], n + bt)
                return outp

            def head(h, xsrc, stg, func, bcol, scol):
                """One head: DoubleRow into [128, BT] psums (rows 0:32 used,
                0:2 valid), both tiles sharing each stationary. Staging rows
                2:31 stay uninitialized — the transpose puts them in columns
                epilogue_post never reads."""
                ph = [phd.tile([128, BT], F32, tag="pm", name=f"ph{h}b{bt}")
                      for bt in range(NBT)]
                if func is AF.Identity:
                    # last head: bt-outer so tile 0's psum completes two
                    # matmuls earlier and its tail chain starts sooner
                    for bt in range(NBT):
                        for t in range(KP5):
                            rhs = xsrc[bt][t].rearrange("p (i b) -> p i b",
                                                        i=2)
                            nc.tensor.matmul(ph[bt][0:32, :],
                                             w5v[:, h, t, :, :], rhs,
                                             start=(t == 0),
                                             stop=(t == KP5 - 1),
                                             perf_mode=DR)
                else:
                    for t in range(KP5):
                        for bt in range(NBT):
                            rhs = xsrc[bt][t].rearrange("p (i b) -> p i b",
                                                        i=2)
                            r = nc.tensor.matmul(ph[bt][0:32, :],
                                                 w5v[:, h, t, :, :], rhs,
                                                 start=(t == 0),
                                                 stop=(t == KP5 - 1),
                                                 perf_mode=DR)
                            if bt > 0:
                                r.ins.ldweights = False
                for bt in range(NBT):
                    dst = stg[0:2, bt * BT:(bt + 1) * BT]
                    if func is AF.Identity and bt == 1:
                        # tile 1 on Vector, tile 0 on the (now idle) Scalar:
                        # the two final head stores run in parallel
                        nc.vector.tensor_scalar(
                            dst, ph[bt][0:2, :], hb[:, scol:scol + 1],
                            hb[:, bcol:bcol + 1], AL.mult, AL.add)
                    else:
                        nc.scalar.activation(
                            dst, ph[bt][0:2, :],
                            AF.Identity if func is AF.Identity else func,
                            bias=hb[:, bcol:bcol + 1],
                            scale=hb[:, scol:scol + 1])

            x1p = layer1()
            pre = epilogue_pre()
            x5a = mp.tile([32, NBT * BT], F32, tag="x5a", name="x5a")
            x5b = mp.tile([32, NBT * BT], F32, tag="x5b", name="x5b")
            vta = mp.tile([32, NBT * BT], F32, tag="vta", name="vta")
            vtb = mp.tile([32, NBT * BT], F32, tag="vtb", name="vtb")

            x2p = dense_dr("x2", x1p, w2v, KP2, N2, BOF["l2"])
            # the whole sigmoid branch runs first: its table swap, head
            # ACTs, transpose, and epilogue_post's opening ops all hide
            # under the identity branch's ~11us of remaining matmuls
            x32p = dense_dr("x32", x2p, w32v, KP3, N3, BOF["l32"])
            x42p = dense_dr("x42", x32p, w42v, KP4, N4, BOF["l42"])
            head(1, x42p, x5b, AF.Sigmoid, 1, 3)
            nc.vector.transpose(vtb, x5b)
            va_all = post_early(vtb, pre)
            x31p = dense_dr("x31", x2p, w31v, KP3, N3, BOF["l31"])
            x41p = dense_dr("x41", x31p, w41v, KP4, N4, BOF["l41"])
            head(0, x41p, x5a, AF.Identity, 0, 2)
            for bt in range(NBT):
                nc.vector.transpose(vta[:, bt * BT:(bt + 1) * BT],
                                    x5a[:, bt * BT:(bt + 1) * BT])
                epilogue_post(bt, vta, va_all, pre)
                nc.sync.dma_start(
                    out=out_d[:, bt * GPB * 2:(bt + 1) * GPB * 2],
                    in_=OUT[:, bt * GPB * 2:(bt + 1) * GPB * 2])

    _shrink_redundant_ldweights(nc)
    nc.compile()
    return nc


def _shrink_redundant_ldweights(nc):
    """The tile legalizer splits every non-f32 matmul into LDWEIGHTS+MATMUL.
    When consecutive PE matmuls share the same stationary (both batch tiles
    per weight chunk), the repeat LDWEIGHTS re-loads identical data; the PE
    weight array persists across matmuls, so shrinking the reload to 16
    columns of the same data is semantically a no-op but ~8x cheaper
    (LDWEIGHTS cost scales with column count)."""
    n_removed = 0
    for b in nc.m.functions[0].blocks:
        insts = b.instructions
        last_sig = None
        to_remove = []
        for idx, inst in enumerate(insts):
            tn = type(inst).__name__
            if tn == 'InstLdweights':
                ap = inst.ins[0]
                dims = [list(p) for p in ap.ap]
                sig = (ap.memref, ap.offset, str(dims))
                if sig == last_sig:
                    # transfer any semaphore waits/updates to the paired
                    # matmul, then drop the load
                    nxt = insts[idx + 1]
                    if type(nxt).__name__ != 'InstMatmult':
                        last_sig = sig
                        continue
                    si = inst.sync_info
                    if si is not None and (si.on_wait or si.on_update):
                        nsi = nxt.sync_info
                        if nsi is None:
                            nxt.sync_info = si
                        else:
                            nxt.sync_info = mybir.SyncInfo(
                                on_wait=list(si.on_wait) + list(nsi.on_wait),
                                on_update=list(si.on_update)
                                + list(nsi.on_update))
                    to_remove.append(inst)
                else:
                    last_sig = sig
            elif tn == 'InstMatmult' and inst.ldweights is not False:
                last_sig = None  # self-loading matmul clobbers the PE array
        for inst in to_remove:
            insts.remove(inst)
            n_removed += 1
    return n_removed


def _q8(a, scale):
    import ml_dtypes
    v = np.clip(np.asarray(a, np.float64) * scale, -240.0, 240.0)
    return v.astype(ml_dtypes.float8_e4m3)


def _pack_pairs(Wq, K, N):
    """[K, N] fp8 -> [128, (K/256)*2*N] with [p, t, i, n] = W[(2t+i)*128+p, n]."""
    return np.ascontiguousarray(
        Wq.reshape(K // 256, 2, 128, N).transpose(2, 0, 1, 3)
        .reshape(128, (K // 256) * 2 * N))


def prep_inputs(x, W1, b1, W2, b2, W31, b31, W32, b32,
                W41, b41, W42, b42, W51, b51, W52, b52):
    """Host-side calibration, quantization, packing -> per-core in_maps."""
    f32 = np.float32
    x = np.asarray(x, f32)
    Ws = {k: np.asarray(v, f32) for k, v in
          dict(W1=W1, W2=W2, W31=W31, W32=W32, W41=W41, W42=W42,
               W51=W51, W52=W52).items()}
    bs = {k: np.asarray(v, f32) for k, v in
          dict(b1=b1, b2=b2, b31=b31, b32=b32, b41=b41, b42=b42,
               b51=b51, b52=b52).items()}

    # calibration forward (fp32) for activation absmax
    relu = lambda v: np.maximum(v, 0.0)
    c1 = relu(x @ Ws["W1"] + bs["b1"])
    c2 = relu(c1 @ Ws["W2"] + bs["b2"])
    c31 = relu(c2 @ Ws["W31"] + bs["b31"])
    c32 = relu(c2 @ Ws["W32"] + bs["b32"])
    c41 = relu(c31 @ Ws["W41"] + bs["b41"])
    c42 = relu(c32 @ Ws["W42"] + bs["b42"])
    amax = {k: max(float(np.abs(v).max()), 1e-6) for k, v in
            dict(x1=c1, x2=c2, x31=c31, x32=c32, x41=c41, x42=c42).items()}
    del c1, c2, c31, c32, c41, c42

    a1 = MARGIN / amax["x1"]

    def beta_for(a_in, amax_out):
        return 2.0 ** np.floor(np.log2((MARGIN / amax_out) / a_in))

    b2s = beta_for(a1, amax["x2"]);      a2 = b2s * a1
    b31s = beta_for(a2, amax["x31"]);    a31 = b31s * a2
    b32s = beta_for(a2, amax["x32"]);    a32 = b32s * a2
    b41s = beta_for(a31, amax["x41"]);   a41 = b41s * a31
    b42s = beta_for(a32, amax["x42"]);   a42 = b42s * a32
    b51s = 192.0 / max(float(np.abs(Ws["W51"]).max()), 1e-6)
    b52s = 192.0 / max(float(np.abs(Ws["W52"]).max()), 1e-6)

    # packed biases [128, 32]: per layer, alpha_out * b reshaped (chunks, 128).T
    bias_pack = np.zeros((128, 32), f32)
    for key, bvec, a_out, nch in [
            ("l1", bs["b1"], a1, N1), ("l2", bs["b2"], a2, N2),
            ("l31", bs["b31"], a31, N3), ("l32", bs["b32"], a32, N3),
            ("l41", bs["b41"], a41, N4), ("l42", bs["b42"], a42, N4)]:
        col = BOF[key]
        bias_pack[:, col:col + nch] = (a_out * bvec).reshape(nch, 128).T

    hbp = np.zeros((2, 4), f32)
    hbp[:, 0] = bs["b51"]
    hbp[:, 1] = bs["b52"]
    hbp[:, 2] = 1.0 / (b51s * a41)
    hbp[:, 3] = 1.0 / (b52s * a42)

    # head weights: pad N 2->32, quantize, pack; concat heads
    def head_pack(Wn, beta):
        Wq = np.zeros((D4, 32), np.float64)
        Wq[:, 0:2] = np.asarray(Wn, np.float64) * beta
        return _pack_pairs(_q8(Wq, 1.0), D4, 32)

    import ml_dtypes
    w5p = np.concatenate(
        [head_pack(Ws["W51"], b51s), head_pack(Ws["W52"], b52s)], axis=1)

    shared = {
        "W2p": _pack_pairs(_q8(Ws["W2"], b2s), D1, D2),
        "W31p": _pack_pairs(_q8(Ws["W31"], b31s), D2, D3),
        "W32p": _pack_pairs(_q8(Ws["W32"], b32s), D2, D3),
        "W41p": _pack_pairs(_q8(Ws["W41"], b41s), D3, D4),
        "W42p": _pack_pairs(_q8(Ws["W42"], b42s), D3, D4),
        "W5p": np.ascontiguousarray(w5p),
        "biasp": bias_pack,
        "hbp": hbp,
    }
    in_maps = []
    for c in range(N_CORES):
        xc = x[c * BC:(c + 1) * BC]
        m = dict(shared)
        m["xw"] = np.ascontiguousarray(
            np.concatenate([xc.T, a1 * Ws["W1"]], axis=1))
        m["Xep"] = np.ascontiguousarray(
            xc.reshape(BC // 32, 32, 5).transpose(1, 0, 2)
            .reshape(32, (BC // 32) * 5))
        in_maps.append(m)
    return in_maps


def unpack_output(results):
    outs = []
    for c in range(N_CORES):
        o = results[c]["out"]  # [32, (BC//32)*2]
        outs.append(o.reshape(32, BC // 32, 2).transpose(1, 0, 2).reshape(BC, 2))
    return np.ascontiguousarray(np.concatenate(outs, axis=0), dtype=np.float32)


_PROG_CACHE = {}


def get_program(consts_key):
    if consts_key not in _PROG_CACHE:
        _PROG_CACHE[consts_key] = build_program(consts_key)
    return _PROG_CACHE[consts_key]


def kernel(x, sgn, mean, std, mean_label, std_label,
           W1, b1, W2, b2, W31, b31, W32, b32,
           W41, b41, W42, b42, W51, b51, W52, b52,
           _trace=False, _tmpdir=None):
    assert int(np.asarray(sgn)) == 1
    consts = (
        tuple(float(v) for v in np.asarray(mean, np.float32)),
        tuple(float(v) for v in np.asarray(std, np.float32)),
        tuple(float(v) for v in np.asarray(mean_label, np.float32)),
        tuple(float(v) for v in np.asarray(std_label, np.float32)),
    )
    nc = get_program(consts)
    in_maps = prep_inputs(x, W1, b1, W2, b2, W31, b31, W32, b32,
                          W41, b41, W42, b42, W51, b51, W52, b52)
    res = run_bass_kernel_spmd(nc, in_maps, core_ids=list(range(N_CORES)),
                               trace=_trace, tmpdir=_tmpdir)
    out = unpack_output(res.results)
    kernel.last_result = res
    return out


# revision 66
# speedup vs baseline: 1.0143x; 1.0143x over previous
"""BarrierNet forward pass on 8 Trainium2 NeuronCores (pure data parallel).

Network (per sample, batch 8192 sharded 1024/core):
  x[5] -> 1024 -> 1024 -> {512, 512} -> {512, 512} -> two 2-wide heads
  followed by a closed-form single-constraint QP projection (dCBF barrier).

v2: fp8(e4m3) DoubleRow matmuls for L2..L4 + heads (~105us -> ~65us vs
the f32r v1). End-to-end rel err ~1.9e-3 (gate 2e-2). Key mechanics:
  - Weights quantized per-tensor with power-of-2 scales chosen so each
    layer's PSUM comes out already in the next layer's storage scale:
    the PSUM->SBUF step is a single add-bias/relu/cast-fp8 instruction
    (Vector tensor_scalar or Scalar activation), no rescale pass.
  - That forces near-unity weight scales; fp8 subnormal storage of the
    uniform-init weights costs ~3.1% rms vs 2.7% at full range (the PE
    widens fp8 to e6m3 internally, honoring subnormals — HW-verified).
  - L1 stays f32r (K=5; bf16/fp8-DR are SLOWER there) with alpha1 folded
    into W1 host-side; activation scales calibrated from a host fp32
    forward of the actual batch (margin 192/240 — never clips to Inf).
  - Layer-major schedule: both 512-sample batch tiles run per weight
    chunk so each stationary is loaded once; the tile-1 LDWEIGHTS that
    the legalizer emits anyway is deleted post-schedule (the PE weight
    array persists across matmuls).
  - The sigmoid branch (L32/L42/head) runs before the identity branch so
    its ACT-table swap (sin and sigmoid share no table) and head output
    processing hide under ~11us of remaining matmuls.
  - Heads run DoubleRow (weights padded to 32 cols); the un-scaling
    1/(beta5*alpha4) enters via ACT scale APs, so the compiled program
    has no data-dependent immediates.
Layout per core: feature-major [feat, batch] tiles, BT=512 batch tiles,
DVE 32x32 stream-transpose for the heads, QP/barrier epilogue on
[32, 32] strided views (batched across both tiles) on Vector/Scalar.
"""

import numpy as np

import concourse.bass as bass
import concourse.tile as tile
from concourse import bacc, mybir
from concourse.bass_utils import run_bass_kernel_spmd

N_CORES = 8
B_FULL = 8192
BC = B_FULL // N_CORES      # batch per core
BT = 512                    # batch tile (matmul moving free dim)
NBT = BC // BT              # batch tiles per core
GPB = BT // 32              # 32-sample groups per batch tile (16)

D1, D2, D3, D4 = 1024, 1024, 512, 512
KP2, KP3, KP4, KP5 = D1 // 256, D2 // 256, D3 // 256, D4 // 256  # k-pair counts
N1, N2, N3, N4 = D1 // 128, D2 // 128, D3 // 128, D4 // 128      # out chunks
L1C, L2C, OBS_X, OBS_Y, RADIUS = 3.0, 3.0, 0.0, 7.0, 4.0

MARGIN = 192.0              # fp8 activation headroom (max normal 240)

F32 = mybir.dt.float32
F32R = mybir.dt.float32r
FP8 = mybir.dt.float8e4
AF = mybir.ActivationFunctionType
AL = mybir.AluOpType
DR = mybir.MatmulPerfMode.DoubleRow

# bias_pack column offsets per layer
BOF = {"l1": 0, "l2": 8, "l31": 16, "l32": 20, "l41": 24, "l42": 28}


def build_program(consts):
    """Build the SPMD Bass program.
    consts = (mean[5], std[5], ml[2], sl[2])."""
    mean, std, ml, sl = consts

    nc = bacc.Bacc("TRN2", target_bir_lowering=False, debug=False,
                   num_devices=N_CORES)

    def din(name, shape, dt):
        return nc.dram_tensor(name, shape, dt, kind="ExternalInput").ap()

    xw_d = din("xw", [5, BC + D1], F32R)  # xT and alpha1*W1 in one DMA
    W2_d = din("W2p", [128, KP2 * 2 * D2], FP8)
    W31_d = din("W31p", [128, KP3 * 2 * D3], FP8)
    W32_d = din("W32p", [128, KP3 * 2 * D3], FP8)
    W41_d = din("W41p", [128, KP4 * 2 * D4], FP8)
    W42_d = din("W42p", [128, KP4 * 2 * D4], FP8)
    W5_d = din("W5p", [128, 2 * KP5 * 2 * 32], FP8)
    Xep_d = din("Xep", [32, NBT * GPB * 5], F32)
    bias_d = din("biasp", [128, 32], F32)
    hb_d = din("hbp", [2, 4], F32)   # cols: b51, b52, 1/(b51sc), 1/(b52sc)
    out_d = nc.dram_tensor("out", [32, NBT * GPB * 2], F32,
                           kind="ExternalOutput").ap()

    with tile.TileContext(nc) as tc:
        with (
            tc.tile_pool(name="wpool", bufs=1) as wp,
            tc.tile_pool(name="acts", bufs=28) as ap_,
            tc.tile_pool(name="misc", bufs=1) as mp,
            tc.tile_pool(name="ep", bufs=1) as ep,
            tc.tile_pool(name="pmm", bufs=8, space="PSUM") as pmm,
        ):
            # ---- input/weight loads -------------------------------------
            # small L1 operands + biases first on the sync ring so the first
            # matmul isn't queued behind the 1MB W2 transfer
            def sync_load(dram, shape, tg, dt=FP8, pool=None):
                t = (pool or wp).tile(shape, dt, tag=tg, name=f"{tg}_t")
                nc.sync.dma_start(out=t, in_=dram)
                return t

            xw = sync_load(xw_d, [5, BC + D1], "xw", F32R, pool=mp)
            xT, w1 = xw[:, 0:BC], xw[:, BC:BC + D1]
            biasp = sync_load(bias_d, [128, 32], "biasp", F32, pool=mp)
            w2 = sync_load(W2_d, [128, KP2 * 2 * D2], "w2")
            w31 = sync_load(W31_d, [128, KP3 * 2 * D3], "w31")
            w32 = sync_load(W32_d, [128, KP3 * 2 * D3], "w32")
            w41 = sync_load(W41_d, [128, KP4 * 2 * D4], "w41")
            w42 = sync_load(W42_d, [128, KP4 * 2 * D4], "w42")

            def gp_load(dram, shape, tg, dt=F32):
                t = mp.tile(shape, dt, tag=tg, name=f"{tg}_t")
                nc.gpsimd.dma_start(out=t, in_=dram)
                return t

            Xep = gp_load(Xep_d, [32, NBT * GPB * 5], "Xep")
            w5 = gp_load(W5_d, [128, 2 * KP5 * 2 * 32], "w5", FP8)
            hb = gp_load(hb_d, [2, 4], "hb")
            phd = pmm  # heads share the 8-bank PSUM pool

            OUT = mp.tile([32, NBT * GPB * 2], F32, tag="OUT", name="OUT_t")

            # PE p-state warmup: the PE idles ~7.2-10.5us waiting for the
            # first input DMA, then runs L1 at cold-clock (~1.7x slow).
            # Dummy matmuls on zeroed tiles (no DMA dependency) fill the
            # idle window and ramp the clock before real work arrives.
            wj = mp.tile([128, 2 * 16], FP8, tag="wj", name="wj_t")
            aj = mp.tile([128, 2 * BT], FP8, tag="aj", name="aj_t")
            nc.vector.memset(wj, 0.0)
            nc.vector.memset(aj, 0.0)
            wjv = wj.rearrange("p (i c) -> p i c", i=2)
            ajv = aj.rearrange("p (i b) -> p i b", i=2)
            for k in range(8):
                psj = pmm.tile([128, BT], F32, tag="pm", name=f"warm{k}")
                nc.tensor.matmul(psj[0:16, :], wjv, ajv, start=True,
                                 stop=True, perf_mode=DR)

            # weight views: [128, pairs, plane, N]
            w2v = w2.rearrange("p (t i n) -> p t i n", t=KP2, i=2)
            w31v = w31.rearrange("p (t i n) -> p t i n", t=KP3, i=2)
            w32v = w32.rearrange("p (t i n) -> p t i n", t=KP3, i=2)
            w41v = w41.rearrange("p (t i n) -> p t i n", t=KP4, i=2)
            w42v = w42.rearrange("p (t i n) -> p t i n", t=KP4, i=2)
            w5v = w5.rearrange("p (h t i c) -> p h t i c", h=2, t=KP5, i=2)

            _cbias_cache = {}

            def cbias(val, parts):
                val = float(val)
                if val not in _cbias_cache:
                    t = ep.tile([128, 1], F32, tag=f"cb{len(_cbias_cache)}",
                                name=f"cb{len(_cbias_cache)}")
                    nc.vector.memset(t, val)
                    _cbias_cache[val] = t
                return _cbias_cache[val][0:parts, :]

            def eact(out, in_, func, bias=0.0, scale=1.0):
                if isinstance(bias, float) and func not in (AF.Copy,):
                    bias = cbias(bias, in_.shape[0])
                nc.scalar.activation(out, in_, func, bias=bias, scale=scale)

            def store_act(dst, ps, bcol, n):
                """dst(fp8) = relu(psum + bias): single instruction. The two
                batch tiles of a chunk go to different engines so the psum
                pair drains in parallel."""
                if n % 2 == 0:
                    nc.vector.tensor_scalar(dst, ps, bcol, 0.0, AL.add, AL.max)
                else:
                    nc.scalar.activation(dst, ps, AF.Relu, bias=bcol)

            HPI = float(np.pi / 2)
            PI = float(np.pi)

            def epilogue_pre():
                """x-only QP/barrier quantities for ALL batch tiles at once
                ([32, NBT*GPB] ops); runs on Vector/Scalar while the PE is
                in the dense layers."""
                NF = NBT * GPB
                Xv = Xep.rearrange("p (f j) -> p f j", j=5)

                def T(nm):
                    return ep.tile([32, NF], F32, tag=nm, bufs=1,
                                   name=f"{nm}_pre")

                def emul(o, a, b):
                    nc.vector.tensor_mul(o, a, b)

                def eadd(o, a, b):
                    nc.vector.tensor_add(o, a, b)

                def stt(o, a, s, op0, b, op1):
                    nc.vector.scalar_tensor_tensor(o, a, float(s), b, op0, op1)

                t1r, w1r = Xv[:, :, 0], Xv[:, :, 1]
                t2r, w2r = Xv[:, :, 2], Xv[:, :, 3]

                if float(std[0]) == 1.0 and float(mean[0]) == 0.0:
                    t1m = t1r
                else:
                    t1m = T("t1m"); eact(t1m, t1r, AF.Copy, bias=float(mean[0]), scale=float(std[0]))
                if float(std[2]) == 1.0 and float(mean[2]) == 0.0:
                    t2m = t2r
                else:
                    t2m = T("t2m"); eact(t2m, t2r, AF.Copy, bias=float(mean[2]), scale=float(std[2]))

                def sincos(theta, nm):
                    ws = T(nm + "_ws"); nc.vector.add_range_wrap(ws, theta, 0.0, PI, 2 * PI)
                    s = T(nm + "_s"); eact(s, ws, AF.Sin)
                    wc = T(nm + "_wc"); nc.vector.add_range_wrap(wc, theta, HPI, PI, 2 * PI)
                    c = T(nm + "_c"); eact(c, wc, AF.Sin)
                    return s, c

                s1, c1 = sincos(t1m, "t1")
                s2, c2 = sincos(t2m, "t2")

                if float(std[1]) == 1.0 and float(mean[1]) == 0.0:
                    w1v_ = w1r
                else:
                    w1v_ = T("w1v"); eact(w1v_, w1r, AF.Copy, bias=float(mean[1]), scale=float(std[1]))
                if float(std[3]) == 1.0 and float(mean[3]) == 0.0:
                    w2v_ = w2r
                else:
                    w2v_ = T("w2v"); eact(w2v_, w2r, AF.Copy, bias=float(mean[3]), scale=float(std[3]))

                pxu = T("pxu"); eadd(pxu, c1, c2)
                px = T("px"); eact(px, pxu, AF.Copy, bias=-OBS_X, scale=L1C)
                pyu = T("pyu"); eadd(pyu, s1, s2)
                py = T("py"); eact(py, pyu, AF.Copy, bias=-OBS_Y, scale=L1C)

                a1 = T("a1"); emul(a1, s1, w1v_)
                a2 = T("a2"); emul(a2, s2, w2v_)
                vxn = T("vxn"); eadd(vxn, a1, a2)          # = -vx/3
                bb1 = T("bb1"); emul(bb1, c1, w1v_)
                bb2 = T("bb2"); emul(bb2, c2, w2v_)
                vyu = T("vyu"); eadd(vyu, bb1, bb2)
                vy = T("vy"); eact(vy, vyu, AF.Copy, scale=3.0)

                q1 = T("q1"); emul(q1, px, vxn)
                q2 = T("q2"); emul(q2, py, vy)
                bdot2 = T("bdot2"); stt(bdot2, q1, -3.0, AL.mult, q2, AL.add)

                w1sq = T("w1sq"); emul(w1sq, w1v_, w1v_)
                w2sq = T("w2sq"); emul(w2sq, w2v_, w2v_)
                cw1 = T("cw1"); emul(cw1, c1, w1sq)
                cw2 = T("cw2"); emul(cw2, c2, w2sq)
                cw = T("cw"); eadd(cw, cw1, cw2)
                sw1 = T("sw1"); emul(sw1, s1, w1sq)
                sw2 = T("sw2"); emul(sw2, s2, w2sq)
                sw = T("sw"); eadd(sw, sw1, sw2)
                t1x = T("t1x"); emul(t1x, px, cw)
                t2y = T("t2y"); emul(t2y, py, sw)
                txy = T("txy"); eadd(txy, t1x, t2y)
                vv1 = T("vv1"); emul(vv1, vxn, vxn)
                vv2 = T("vv2"); emul(vv2, vy, vy)
                vv = T("vv"); stt(vv, vv1, 9.0, AL.mult, vv2, AL.add)
                Lhalf = T("Lhalf"); stt(Lhalf, txy, -3.0, AL.mult, vv, AL.add)

                g1a = T("g1a"); emul(g1a, px, s1)
                g1b = T("g1b"); emul(g1b, py, c1)
                g2a = T("g2a"); emul(g2a, px, s2)
                g2b = T("g2b"); emul(g2b, py, c2)
                G12 = ep.tile([32, NF * 2], F32, tag="G12", bufs=1,
                              name="G12_pre")
                G12v = G12.rearrange("p (f q) -> p f q", q=2)
                G1h, G2h = G12v[:, :, 0], G12v[:, :, 1]
                stt(G1h, g1b, -1.0, AL.mult, g1a, AL.add)  # G1/6
                stt(G2h, g2b, -1.0, AL.mult, g2a, AL.add)  # G2/6

                pxsq = T("pxsq"); emul(pxsq, px, px)
                pysq = T("pysq"); emul(pysq, py, py)
                bar = T("bar"); stt(bar, pxsq, -RADIUS * RADIUS, AL.add, pysq, AL.add)

                d1 = T("d1"); emul(d1, G1h, G1h)
                d2 = T("d2"); emul(d2, G2h, G2h)
                den36 = T("den36"); stt(den36, d1, 1e-12 / 36.0, AL.add, d2, AL.add)
                nrec = T("nrec"); nc.vector.reciprocal(nrec, den36)

                return dict(bdot2=bdot2, bar=bar, Lhalf=Lhalf,
                            G1h=G1h, G2h=G2h, G12=G12, nrec=nrec)

            def post_early(vtb, pre):
                """Sigmoid-dependent half of the QP tail, both tiles at once
                ([32, 32] ops): runs under the identity branch's matmuls."""
                NF = NBT * GPB
                Yvb = vtb.rearrange("p (f q) -> p f q", q=32)
                sg1, sg2 = Yvb[:, :, 0], Yvb[:, :, 1]

                def T(nm):
                    return ep.tile([32, NF], F32, tag=nm, bufs=1,
                                   name=f"{nm}_pearly")

                ssum = T("ssum"); nc.vector.tensor_add(ssum, sg1, sg2)
                sprod = T("sprod"); nc.vector.tensor_mul(sprod, sg1, sg2)
                hb_ = T("hb_"); nc.vector.tensor_mul(hb_, ssum, pre["bdot2"])
                hc = T("hc"); nc.vector.tensor_mul(hc, sprod, pre["bar"])
                va2 = T("va2"); nc.vector.scalar_tensor_tensor(
                    va2, hc, 8.0, pre["Lhalf"], AL.mult, AL.add)
                va = T("va"); nc.vector.scalar_tensor_tensor(
                    va, hb_, 4.0, va2, AL.mult, AL.add)  # h/2
                return va

            def epilogue_post(bt, vta, va_all, pre):
                """Identity-head-dependent tail of the QP for batch tile bt."""
                fsl = slice(bt * GPB, (bt + 1) * GPB)
                Yva = vta.rearrange("p (f q) -> p f q", q=32)[:, fsl, :]
                OUTv = OUT.rearrange("p (f i) -> p f i", i=2)[:, fsl, :]

                def T(nm):
                    return ep.tile([32, GPB], F32, tag=nm, bufs=NBT,
                                   name=f"{nm}_post{bt}")

                def emul(o, a, b):
                    nc.vector.tensor_mul(o, a, b)

                def eadd(o, a, b):
                    nc.vector.tensor_add(o, a, b)

                def stt(o, a, s, op0, b, op1):
                    nc.vector.scalar_tensor_tensor(o, a, float(s), b, op0, op1)

                nrec = pre["nrec"][:, fsl]
                va = va_all[:, fsl]
                G12s = pre["G12"].rearrange("p (f q) -> p f q", q=2)[:, fsl, :]
                P12 = Yva[:, :, 0:2]  # [32, GPB, 2]

                r12 = ep.tile([32, GPB * 2], F32, tag="r12", bufs=NBT,
                              name=f"r12_post{bt}")
                r12v = r12.rearrange("p (f q) -> p f q", q=2)
                nc.vector.tensor_mul(r12v, G12s, P12)
                rs = T("rs"); eadd(rs, r12v[:, :, 0], r12v[:, :, 1])
                vb = T("vb"); stt(vb, rs, 3.0, AL.mult, va, AL.add)    # viol=-2vb

                vr = T("vr")
                nc.vector.tensor_scalar(vr, vb, -1.0, 0.0, AL.mult, AL.max)
                lam18 = T("lam18"); emul(lam18, vr, nrec)

                lam18b = bass.AP(tensor=lam18.tensor, offset=lam18.offset,
                                 ap=list(lam18.ap) + [[0, 2]])
                lg12 = ep.tile([32, GPB * 2], F32, tag="lg12", bufs=NBT,
                               name=f"lg12_post{bt}")
                lg12v = lg12.rearrange("p (f q) -> p f q", q=2)
                nc.vector.tensor_mul(lg12v, lam18b, G12s)
                if (float(sl[0]) == 1.0 and float(sl[1]) == 1.0
                        and float(ml[0]) == 0.0 and float(ml[1]) == 0.0):
                    # out = -(lg12/3 + P12): one DVE op straight into OUT
                    stt(OUTv[:, :, 0:2], lg12v, -1.0 / 3.0, AL.mult, P12,
                        AL.subtract)
                else:
                    u12n = ep.tile([32, GPB * 2], F32, tag="u12n", bufs=NBT,
                                   name=f"u12n_post{bt}")
                    u12v = u12n.rearrange("p (f q) -> p f q", q=2)
                    stt(u12v, lg12v, 1.0 / 3.0, AL.mult, P12, AL.add)
                    eact(OUTv[:, :, 0], u12v[:, :, 0], AF.Copy,
                         bias=-float(ml[0]) / float(sl[0]),
                         scale=-1.0 / float(sl[0]))
                    eact(OUTv[:, :, 1], u12v[:, :, 1], AF.Copy,
                         bias=-float(ml[1]) / float(sl[1]),
                         scale=-1.0 / float(sl[1]))

            def pair_tiles(nm, n_pairs, bt):
                return [ap_.tile([128, 2 * BT], FP8, tag="act",
                                 name=f"{nm}_p{t}b{bt}")
                        for t in range(n_pairs)]

            def layer1():
                """L1 (f32r, K=5, alpha1 pre-folded into W1) -> fp8 pairs.
                Tile-outer so tile 0's stores drain while tile 1's matmuls
                run — L2 (which needs ALL of a tile's x1) starts sooner."""
                x1p = [pair_tiles("x1", N1 // 2, bt) for bt in range(NBT)]
                for bt in range(NBT):
                    for n in range(N1):
                        ps = pmm.tile([128, BT], F32, tag="pm",
                                      name=f"ps1_{n}b{bt}")
                        nc.tensor.matmul(
                            ps, w1[:, n * 128:(n + 1) * 128],
                            xT[:, bt * BT:(bt + 1) * BT], start=True,
                            stop=True)
                        store_act(
                            x1p[bt][n // 2][:, (n % 2) * BT:(n % 2 + 1) * BT],
                            ps, biasp[:, BOF["l1"] + n:BOF["l1"] + n + 1], n)
                return x1p

            def dense_dr(nm, inp, wv, n_pairs_k, n_out, bof):
                """fp8 DoubleRow dense layer, both batch tiles per stationary
                (tile 1 reuses the loaded weights: ldweights=False)."""
                outp = [pair_tiles(nm, n_out // 2, bt) for bt in range(NBT)]
                for n in range(n_out):
                    ps = [pmm.tile([128, BT], F32, tag="pm",
                                   name=f"ps{nm}_{n}b{bt}")
                          for bt in range(NBT)]
                    for t in range(n_pairs_k):
                        for bt in range(NBT):
                            rhs = inp[bt][t].rearrange("p (i b) -> p i b", i=2)
                            r = nc.tensor.matmul(
                                ps[bt], wv[:, t, :, n * 128:(n + 1) * 128],
                                rhs, start=(t == 0),
                                stop=(t == n_pairs_k - 1), perf_mode=DR)
                            if bt > 0:
                                r.ins.ldweights = False
                    for bt in range(NBT):
                        store_act(
                            outp[bt][n // 2][:, (n % 2) * BT:(n % 2 + 1) * BT],
                            ps[bt], biasp[:, bof + n:bof + n + 1], n + bt)
                return outp

            def head(h, xsrc, stg, func, bcol, scol):
                """One head: DoubleRow into [128, BT] psums (rows 0:32 used,
                0:2 valid), both tiles sharing each stationary. Staging rows
                2:31 stay uninitialized — the transpose puts them in columns
                epilogue_post never reads."""
                ph = [phd.tile([128, BT], F32, tag="pm", name=f"ph{h}b{bt}")
                      for bt in range(NBT)]
                if func is AF.Identity:
                    # last head: bt-outer so tile 0's psum completes two
                    # matmuls earlier and its tail chain starts sooner
                    for bt in range(NBT):
                        for t in range(KP5):
                            rhs = xsrc[bt][t].rearrange("p (i b) -> p i b",
                                                        i=2)
                            nc.tensor.matmul(ph[bt][0:32, :],
                                             w5v[:, h, t, :, :], rhs,
                                             start=(t == 0),
                                             stop=(t == KP5 - 1),
                                             perf_mode=DR)
                else:
                    for t in range(KP5):
                        for bt in range(NBT):
                            rhs = xsrc[bt][t].rearrange("p (i b) -> p i b",
                                                        i=2)
                            r = nc.tensor.matmul(ph[bt][0:32, :],
                                                 w5v[:, h, t, :, :], rhs,
                                                 start=(t == 0),
                                                 stop=(t == KP5 - 1),
                                                 perf_mode=DR)
                            if bt > 0:
                                r.ins.ldweights = False
                for bt in range(NBT):
                    dst = stg[0:2, bt * BT:(bt + 1) * BT]
                    if func is AF.Identity and bt == 1:
                        # tile 1 on Vector, tile 0 on the (now idle) Scalar:
                        # the two final head stores run in parallel
                        nc.vector.tensor_scalar(
                            dst, ph[bt][0:2, :], hb[:, scol:scol + 1],
                            hb[:, bcol:bcol + 1], AL.mult, AL.add)
                    else:
                        nc.scalar.activation(
                            dst, ph[bt][0:2, :],
                            AF.Identity if func is AF.Identity else func,
                            bias=hb[:, bcol:bcol + 1],
                            scale=hb[:, scol:scol + 1])

            x1p = layer1()
            pre = epilogue_pre()
            x5a = mp.tile([32, NBT * BT], F32, tag="x5a", name="x5a")
            x5b = mp.tile([32, NBT * BT], F32, tag="x5b", name="x5b")
            vta = mp.tile([32, NBT * BT], F32, tag="vta", name="vta")
            vtb = mp.tile([32, NBT * BT], F32, tag="vtb", name="vtb")

            x2p = dense_dr("x2", x1p, w2v, KP2, N2, BOF["l2"])
            # the whole sigmoid branch runs first: its table swap, head
            # ACTs, transpose, and epilogue_post's opening ops all hide
            # under the identity branch's ~11us of remaining matmuls
            x32p = dense_dr("x32", x2p, w32v, KP3, N3, BOF["l32"])
            x42p = dense_dr("x42", x32p, w42v, KP4, N4, BOF["l42"])
            head(1, x42p, x5b, AF.Sigmoid, 1, 3)
            nc.vector.transpose(vtb, x5b)
            va_all = post_early(vtb, pre)
            x31p = dense_dr("x31", x2p, w31v, KP3, N3, BOF["l31"])
            x41p = dense_dr("x41", x31p, w41v, KP4, N4, BOF["l41"])
            head(0, x41p, x5a, AF.Identity, 0, 2)
            for bt in range(NBT):
                nc.vector.transpose(vta[:, bt * BT:(bt + 1) * BT],
                                    x5a[:, bt * BT:(bt + 1) * BT])
                epilogue_post(bt, vta, va_all, pre)
                nc.sync.dma_start(
                    out=out_d[:, bt * GPB * 2:(bt + 1) * GPB * 2],
                    in_=OUT[:, bt * GPB * 2:(bt + 1) * GPB * 2])

    _shrink_redundant_ldweights(nc)
    nc.compile()
    return nc


def _shrink_redundant_ldweights(nc):
    """The tile legalizer splits every non-f32 matmul into LDWEIGHTS+MATMUL.
    When consecutive PE matmuls share the same stationary (both batch tiles
    per weight chunk), the repeat LDWEIGHTS re-loads identical data; the PE
    weight array persists across matmuls, so shrinking the reload to 16
    columns of the same data is semantically a no-op but ~8x cheaper
    (LDWEIGHTS cost scales with column count)."""
    n_removed = 0
    for b in nc.m.functions[0].blocks:
        insts = b.instructions
        last_sig = None
        to_remove = []
        for idx, inst in enumerate(insts):
            tn = type(inst).__name__
            if tn == 'InstLdweights':
                ap = inst.ins[0]
                dims = [list(p) for p in ap.ap]
                sig = (ap.memref, ap.offset, str(dims))
                if sig == last_sig:
                    # transfer any semaphore waits/updates to the paired
                    # matmul, then drop the load
                    nxt = insts[idx + 1]
                    if type(nxt).__name__ != 'InstMatmult':
                        last_sig = sig
                        continue
                    si = inst.sync_info
                    if si is not None and (si.on_wait or si.on_update):
                        nsi = nxt.sync_info
                        if nsi is None:
                            nxt.sync_info = si
                        else:
                            nxt.sync_info = mybir.SyncInfo(
                                on_wait=list(si.on_wait) + list(nsi.on_wait),
                                on_update=list(si.on_update)
                                + list(nsi.on_update))
                    to_remove.append(inst)
                else:
                    last_sig = sig
            elif tn == 'InstMatmult' and inst.ldweights is not False:
                last_sig = None  # self-loading matmul clobbers the PE array
        for inst in to_remove:
            insts.remove(inst)
            n_removed += 1
    return n_removed


def _q8(a, scale):
    import ml_dtypes
    v = np.clip(np.asarray(a, np.float64) * scale, -240.0, 240.0)
    return v.astype(ml_dtypes.float8_e4m3)


def _pack_pairs(Wq, K, N):
    """[K, N] fp8 -> [128, (K/256)*2*N] with [p, t, i, n] = W[(2t+i)*128+p, n]."""
    return np.ascontiguousarray(
        Wq.reshape(K // 256, 2, 128, N).transpose(2, 0, 1, 3)
        .reshape(128, (K // 256) * 2 * N))


def prep_inputs(x, W1, b1, W2, b2, W31, b31, W32, b32,
                W41, b41, W42, b42, W51, b51, W52, b52):
    """Host-side calibration, quantization, packing -> per-core in_maps."""
    f32 = np.float32
    x = np.asarray(x, f32)
    Ws = {k: np.asarray(v, f32) for k, v in
          dict(W1=W1, W2=W2, W31=W31, W32=W32, W41=W41, W42=W42,
               W51=W51, W52=W52).items()}
    bs = {k: np.asarray(v, f32) for k, v in
          dict(b1=b1, b2=b2, b31=b31, b32=b32, b41=b41, b42=b42,
               b51=b51, b52=b52).items()}

    # calibration forward (fp32) for activation absmax
    relu = lambda v: np.maximum(v, 0.0)
    c1 = relu(x @ Ws["W1"] + bs["b1"])
    c2 = relu(c1 @ Ws["W2"] + bs["b2"])
    c31 = relu(c2 @ Ws["W31"] + bs["b31"])
    c32 = relu(c2 @ Ws["W32"] + bs["b32"])
    c41 = relu(c31 @ Ws["W41"] + bs["b41"])
    c42 = relu(c32 @ Ws["W42"] + bs["b42"])
    amax = {k: max(float(np.abs(v).max()), 1e-6) for k, v in
            dict(x1=c1, x2=c2, x31=c31, x32=c32, x41=c41, x42=c42).items()}
    del c1, c2, c31, c32, c41, c42

    a1 = MARGIN / amax["x1"]

    def beta_for(a_in, amax_out):
        return 2.0 ** np.floor(np.log2((MARGIN / amax_out) / a_in))

    b2s = beta_for(a1, amax["x2"]);      a2 = b2s * a1
    b31s = beta_for(a2, amax["x31"]);    a31 = b31s * a2
    b32s = beta_for(a2, amax["x32"]);    a32 = b32s * a2
    b41s = beta_for(a31, amax["x41"]);   a41 = b41s * a31
    b42s = beta_for(a32, amax["x42"]);   a42 = b42s * a32
    b51s = 192.0 / max(float(np.abs(Ws["W51"]).max()), 1e-6)
    b52s = 192.0 / max(float(np.abs(Ws["W52"]).max()), 1e-6)

    # packed biases [128, 32]: per layer, alpha_out * b reshaped (chunks, 128).T
    bias_pack = np.zeros((128, 32), f32)
    for key, bvec, a_out, nch in [
            ("l1", bs["b1"], a1, N1), ("l2", bs["b2"], a2, N2),
            ("l31", bs["b31"], a31, N3), ("l32", bs["b32"], a32, N3),
            ("l41", bs["b41"], a41, N4), ("l42", bs["b42"], a42, N4)]:
        col = BOF[key]
        bias_pack[:, col:col + nch] = (a_out * bvec).reshape(nch, 128).T

    hbp = np.zeros((2, 4), f32)
    hbp[:, 0] = bs["b51"]
    hbp[:, 1] = bs["b52"]
    hbp[:, 2] = 1.0 / (b51s * a41)
    hbp[:, 3] = 1.0 / (b52s * a42)

    # head weights: pad N 2->32, quantize, pack; concat heads
    def head_pack(Wn, beta):
        Wq = np.zeros((D4, 32), np.float64)
        Wq[:, 0:2] = np.asarray(Wn, np.float64) * beta
        return _pack_pairs(_q8(Wq, 1.0), D4, 32)

    import ml_dtypes
    w5p = np.concatenate(
        [head_pack(Ws["W51"], b51s), head_pack(Ws["W52"], b52s)], axis=1)

    shared = {
        "W2p": _pack_pairs(_q8(Ws["W2"], b2s), D1, D2),
        "W31p": _pack_pairs(_q8(Ws["W31"], b31s), D2, D3),
        "W32p": _pack_pairs(_q8(Ws["W32"], b32s), D2, D3),
        "W41p": _pack_pairs(_q8(Ws["W41"], b41s), D3, D4),
        "W42p": _pack_pairs(_q8(Ws["W42"], b42s), D3, D4),
        "W5p": np.ascontiguousarray(w5p),
        "biasp": bias_pack,
        "hbp": hbp,
    }
    in_maps = []
    for c in range(N_CORES):
        xc = x[c * BC:(c + 1) * BC]
        m = dict(shared)
        m["xw"] = np.ascontiguousarray(
            np.concatenate([xc.T, a1 * Ws["W1"]], axis=1))
        m["Xep"] = np.ascontiguousarray(
            xc.reshape(BC // 32, 32, 5).transpose(1, 0, 2)
            .reshape(32, (BC // 32) * 5))
        in_maps.append(m)
    return in_maps


def unpack_output(results):
    outs = []
    for c in range(N_CORES):
        o = results[c]["out"]  # [32, (BC//32)*2]
        outs.append(o.reshape(32, BC // 32, 2).transpose(1, 0, 2).reshape(BC, 2))
    return np.ascontiguousarray(np.concatenate(outs, axis=0), dtype=np.float32)


_PROG_CACHE = {}


def get_program(consts_key):
    if consts_key not in _PROG_CACHE:
        _PROG_CACHE[consts_key] = build_program(consts_key)
    return _PROG_CACHE[consts_key]


def kernel(x, sgn, mean, std, mean_label, std_label,
           W1, b1, W2, b2, W31, b31, W32, b32,
           W41, b41, W42, b42, W51, b51, W52, b52,
           _trace=False, _tmpdir=None):
    assert int(np.asarray(sgn)) == 1
    consts = (
        tuple(float(v) for v in np.asarray(mean, np.float32)),
        tuple(float(v) for v in np.asarray(std, np.float32)),
        tuple(float(v) for v in np.asarray(mean_label, np.float32)),
        tuple(float(v) for v in np.asarray(std_label, np.float32)),
    )
    nc = get_program(consts)
    in_maps = prep_inputs(x, W1, b1, W2, b2, W31, b31, W32, b32,
                          W41, b41, W42, b42, W51, b51, W52, b52)
    res = run_bass_kernel_spmd(nc, in_maps, core_ids=list(range(N_CORES)),
                               trace=_trace, tmpdir=_tmpdir)
    out = unpack_output(res.results)
    kernel.last_result = res
    return out
